# revision 11
# baseline (speedup 1.0000x reference)
"""LoRALinear kernel for Trainium2 (8 NeuronCores, SPMD data-parallel).

Computes out = x @ W.T + b + SCALE*((x@gA.T)@gB.T + (x@lA.T)@lB.T)
  x: [8, 2048, 1024] f32, W: [4096, 1024], b: [4096]
  gA/lA: [8, 1024], gB/lB: [4096, 8]  ->  out: [8, 2048, 4096] f32

Strategy: one batch of x per core. Host marshals pure layout/dtype only
(no module FLOPs): x -> x.T fp16 per core, W -> W.T fp16, b broadcast to
[128, 4096] f32, LoRA adapters stacked/pre-scaled as in the reference
low-rank-first formulation (A_cat = SCALE*[gA;lA] fp16, B_catT =
[gB.T;lB.T] fp16).

Device does all the math in ONE fused pipeline over o-tiles of 512:
  1. merge: wet[k, o] = W.T chunk + A_cat.T @ B_catT chunk (rank-16
     matmul into PSUM, DVE add evicts onto the DMA'd W.T chunk in a
     3-deep ring). The merge of o-tile ot+1 is sprinkled between the
     s-tile groups of main(ot) so neither the PE nor the Vector FIFO
     ever stalls at an o-tile boundary.
  2. main: for each of 16 s-tiles, 8 fp16 matmuls accumulate
     psum[s, o] over k; DVE adds bias f32 during eviction; out DMAs
     ride the Activation DGE queue so input prefetch (Sync queue)
     never queues behind stores.
A short burst of dependency-free warmup matmuls flips the PE HAM clock
gate to full rate while the first DMAs stream in. The PE stream is
~1088 back-to-back N=512 fp16 matmuls (~235us at 2.4GHz) with no phase
boundaries.

fp16 operand rounding gives ~3e-4 absmax relative error vs the f32
reference; accumulation stays f32 in PSUM.
"""
import numpy as np
from contextlib import ExitStack

import concourse.bass as bass
import concourse.tile as tile
from concourse import bacc, mybir
from concourse.bass import ts, ds
from concourse.bass_utils import run_bass_kernel_spmd

F32 = mybir.dt.float32
F16 = mybir.dt.float16

N_CORES = 8
B, S, DIN, DOUT, R = 8, 2048, 1024, 4096, 8
SCALE = 16.0 / 8
R2 = 2 * R

P = 128            # partition tile
OTILE = 512        # matmul moving free dim (one PSUM bank of f32)
KT = DIN // P      # 8 k-tiles
OT = DOUT // OTILE # 8 o-tiles
ST = S // P        # 16 s-tiles
SC = S // OTILE    # 4 s-chunks of 512 for x DMA granularity
WARMUP_MM = 10     # HAM warmup matmuls: bridge PE from preamble (~7.6us)
                   # to first real matmul (~11us) so the stream starts warm


def build_nc():
    nc = bacc.Bacc("TRN2", target_bir_lowering=False, debug=False,
                   num_devices=N_CORES)
    xT = nc.dram_tensor("xT", [DIN, S], F16, kind="ExternalInput").ap()
    WT = nc.dram_tensor("WT", [DIN, DOUT], F16, kind="ExternalInput").ap()
    bias = nc.dram_tensor("bias", [P, DOUT], F32, kind="ExternalInput").ap()
    A_cat = nc.dram_tensor("A_cat", [R2, DIN], F16, kind="ExternalInput").ap()
    B_catT = nc.dram_tensor("B_catT", [R2, DOUT], F16, kind="ExternalInput").ap()
    out = nc.dram_tensor("out", [S, DOUT], F32, kind="ExternalOutput").ap()

    with tile.TileContext(nc) as tc:
        with ExitStack() as ctx:
            const = ctx.enter_context(tc.tile_pool(name="const", bufs=1))
            xt_pool = ctx.enter_context(tc.tile_pool(name="xt", bufs=1))
            wet_pool = ctx.enter_context(tc.tile_pool(name="wet", bufs=3))
            out_pool = ctx.enter_context(tc.tile_pool(name="outp", bufs=4))
            pl_pool = ctx.enter_context(tc.tile_pool(name="pl", bufs=2, space="PSUM"))
            po_pool = ctx.enter_context(tc.tile_pool(name="po", bufs=6, space="PSUM"))

            # ---- HAM warmup: dependency-free matmuls run while the first
            # DMAs stream, so real matmuls start at 2.4GHz, not 1.2 ----
            junk = const.tile([P, OTILE], F16)
            nc.vector.memset(junk[:], 1.0)
            for i in range(WARMUP_MM):
                pw = po_pool.tile([P, OTILE], F32, tag="po")
                nc.tensor.matmul(pw[:], junk[:, :P], junk[:],
                                 start=True, stop=True)

            # ---- constants (input DMAs all ride the Sync DGE queue, in
            # program order; outputs use the Activation DGE queue) ----
            acat = const.tile([R2, DIN], F16)
            nc.sync.dma_start(acat[:], A_cat)
            bcatt = const.tile([R2, DOUT], F16)
            nc.sync.dma_start(bcatt[:], B_catT)
            bias_sb = const.tile([P, DOUT], F32)

            # x.T tiles: [128, k-tile, s-chunk 512] fp16, one DMA per s-chunk
            xts = [xt_pool.tile([P, KT, OTILE], F16, name=f"xt{sc}")
                   for sc in range(SC)]

            def issue_x_dma(sc, eng=None):
                # sc=0 rides the Activation DGE queue so its 1MB transfer
                # runs in parallel with wet0 on the Sync queue at startup
                (eng or nc.sync).dma_start(
                    xts[sc][:],
                    xT[:, ts(sc, OTILE)].rearrange("(kt p) s -> p kt s", p=P))

            def issue_wet_dma(ot):
                # DMA W.T chunk for o-tile `ot` into a fresh ring buffer
                w = wet_pool.tile([P, KT, OTILE], F16, tag="wet",
                                  name=f"wet{ot}")
                nc.sync.dma_start(
                    w[:],
                    WT[:, ts(ot, OTILE)].rearrange("(kt p) o -> p kt o", p=P))
                return w

            def merge_one(wet, ot, kt):
                # merge LoRA into one W.T chunk: wet[kt] += acat.T @ bcatt
                pl = pl_pool.tile([P, OTILE], F32, tag="pl")
                nc.tensor.matmul(pl[:], acat[:, ts(kt, P)],
                                 bcatt[:, ts(ot, OTILE)],
                                 start=True, stop=True)
                nc.vector.tensor_tensor(wet[:, kt, :], wet[:, kt, :], pl[:],
                                        mybir.AluOpType.add)

            wet_cur = issue_wet_dma(0)
            issue_x_dma(0, eng=nc.scalar)
            nc.sync.dma_start(bias_sb[:, ts(0, OTILE)], bias[:, ts(0, OTILE)])
            wet_next = issue_wet_dma(1)
            for sc in range(1, SC):
                issue_x_dma(sc)
            for kt in range(KT):
                merge_one(wet_cur, 0, kt)

            # ---- fused, software-pipelined merge + main loop over o-tiles
            for ot in range(OT):
                if ot + 2 < OT:
                    wet_fut = issue_wet_dma(ot + 2)
                if ot + 1 < OT:
                    nc.sync.dma_start(bias_sb[:, ts(ot + 1, OTILE)],
                                      bias[:, ts(ot + 1, OTILE)])
                # main: out[s, o] = x @ wet + bias
                for st in range(ST):
                    if st % 2 == 1 and ot + 1 < OT:
                        merge_one(wet_next, ot + 1, (st - 1) // 2)
                    sc, sp = st // 4, st % 4
                    po = po_pool.tile([P, OTILE], F32, tag="po")
                    for kt in range(KT):
                        nc.tensor.matmul(po[:], xts[sc][:, kt, ts(sp, P)],
                                         wet_cur[:, kt, :],
                                         start=(kt == 0), stop=(kt == KT - 1))
                    osb = out_pool.tile([P, OTILE], F32)
                    nc.vector.tensor_tensor(osb[:], po[:],
                                            bias_sb[:, ts(ot, OTILE)],
                                            mybir.AluOpType.add)
                    nc.scalar.dma_start(out[ts(st, P), ts(ot, OTILE)], osb[:])
                if ot + 1 < OT:
                    wet_cur = wet_next
                if ot + 2 < OT:
                    wet_next = wet_fut

    nc.compile()
    return nc


_NC_CACHE = None


def _get_nc():
    global _NC_CACHE
    if _NC_CACHE is None:
        _NC_CACHE = build_nc()
    return _NC_CACHE


def make_in_maps(x, W, b, global_A, global_B, local_A, local_B):
    x16 = np.asarray(x, dtype=np.float32).astype(np.float16)
    xT = np.ascontiguousarray(x16.transpose(0, 2, 1))          # [B, DIN, S]
    WT = np.ascontiguousarray(
        np.asarray(W, dtype=np.float32).T).astype(np.float16)  # [DIN, DOUT]
    bias = np.ascontiguousarray(
        np.broadcast_to(np.asarray(b, dtype=np.float32), (P, DOUT)))
    A_cat = np.ascontiguousarray(
        SCALE * np.concatenate([np.asarray(global_A), np.asarray(local_A)],
                               axis=0)).astype(np.float16)
    B_catT = np.ascontiguousarray(
        np.concatenate([np.asarray(global_B).T, np.asarray(local_B).T],
                       axis=0)).astype(np.float16)
    return [
        {"xT": xT[i], "WT": WT, "bias": bias, "A_cat": A_cat,
         "B_catT": B_catT}
        for i in range(N_CORES)
    ]


def kernel(x, W, b, global_A, global_B, local_A, local_B):
    nc = _get_nc()
    in_maps = make_in_maps(x, W, b, global_A, global_B, local_A, local_B)
    res = run_bass_kernel_spmd(nc, in_maps, list(range(N_CORES))).results
    return np.stack([res[i]["out"] for i in range(N_CORES)], axis=0)


# revision 15
# speedup vs baseline: 1.0044x; 1.0044x over previous
"""LoRALinear kernel for Trainium2 (8 NeuronCores, SPMD data-parallel).

Computes out = x @ W.T + b + SCALE*((x@gA.T)@gB.T + (x@lA.T)@lB.T)
  x: [8, 2048, 1024] f32, W: [4096, 1024], b: [4096]
  gA/lA: [8, 1024], gB/lB: [4096, 8]  ->  out: [8, 2048, 4096] f32

Strategy: one batch of x per core. Host marshals pure layout/dtype only
(no module FLOPs): x -> x.T fp16 per core, W -> W.T fp16, b broadcast to
[128, 4096] f32, LoRA adapters stacked/pre-scaled as in the reference
low-rank-first formulation (A_cat = SCALE*[gA;lA] fp16, B_catT =
[gB.T;lB.T] fp16).

Device does all the math in ONE fused pipeline over o-tiles of 512:
  1. merge: wet[k, o] = W.T chunk + A_cat.T @ B_catT chunk (rank-16
     matmul into PSUM, DVE add evicts onto the DMA'd W.T chunk in a
     3-deep ring). The merge of o-tile ot+1 is sprinkled between the
     s-tile groups of main(ot) so neither the PE nor the Vector FIFO
     ever stalls at an o-tile boundary.
  2. main: for each of 16 s-tiles, 8 fp16 matmuls accumulate
     psum[s, o] over k; DVE adds bias f32 during eviction; out DMAs
     ride the Activation DGE queue so input prefetch (Sync queue)
     never queues behind stores.
A short burst of dependency-free warmup matmuls flips the PE HAM clock
gate to full rate while the first DMAs stream in. The PE stream is
~1088 back-to-back N=512 fp16 matmuls (~235us at 2.4GHz) with no phase
boundaries.

fp16 operand rounding gives ~3e-4 absmax relative error vs the f32
reference; accumulation stays f32 in PSUM.
"""
import numpy as np
from contextlib import ExitStack

import concourse.bass as bass
import concourse.tile as tile
from concourse import bacc, mybir
from concourse.bass import ts, ds
from concourse.bass_utils import run_bass_kernel_spmd

F32 = mybir.dt.float32
F16 = mybir.dt.float16

N_CORES = 8
B, S, DIN, DOUT, R = 8, 2048, 1024, 4096, 8
SCALE = 16.0 / 8
R2 = 2 * R

P = 128            # partition tile
OTILE = 512        # matmul moving free dim (one PSUM bank of f32)
KT = DIN // P      # 8 k-tiles
OT = DOUT // OTILE # 8 o-tiles
ST = S // P        # 16 s-tiles
SC = S // OTILE    # 4 s-chunks of 512 for x DMA granularity
WARMUP_MM = 8      # HAM warmup matmuls: bridge PE from preamble (~7.6us)
                   # to first real matmul (~9us) so the stream starts warm


def build_nc():
    nc = bacc.Bacc("TRN2", target_bir_lowering=False, debug=False,
                   num_devices=N_CORES)
    xT = nc.dram_tensor("xT", [DIN, S], F16, kind="ExternalInput").ap()
    WT = nc.dram_tensor("WT", [DIN, DOUT], F16, kind="ExternalInput").ap()
    bias = nc.dram_tensor("bias", [P, DOUT], F32, kind="ExternalInput").ap()
    A_cat = nc.dram_tensor("A_cat", [R2, DIN], F16, kind="ExternalInput").ap()
    B_catT = nc.dram_tensor("B_catT", [R2, DOUT], F16, kind="ExternalInput").ap()
    out = nc.dram_tensor("out", [S, DOUT], F32, kind="ExternalOutput").ap()

    with tile.TileContext(nc) as tc:
        with ExitStack() as ctx:
            const = ctx.enter_context(tc.tile_pool(name="const", bufs=1))
            xt_pool = ctx.enter_context(tc.tile_pool(name="xt", bufs=1))
            wet_pool = ctx.enter_context(tc.tile_pool(name="wet", bufs=3))
            out_pool = ctx.enter_context(tc.tile_pool(name="outp", bufs=4))
            pl_pool = ctx.enter_context(tc.tile_pool(name="pl", bufs=2, space="PSUM"))
            po_pool = ctx.enter_context(tc.tile_pool(name="po", bufs=6, space="PSUM"))

            # ---- HAM warmup: dependency-free matmuls run while the first
            # DMAs stream, so real matmuls start at 2.4GHz, not 1.2 ----
            junk = const.tile([P, OTILE], F16)
            nc.vector.memset(junk[:], 1.0)
            for i in range(WARMUP_MM):
                pw = po_pool.tile([P, OTILE], F32, tag="po")
                nc.tensor.matmul(pw[:], junk[:, :P], junk[:],
                                 start=True, stop=True)

            # ---- constants. Startup latency is DMA-bound: spread the
            # first-needed transfers across BOTH DGE queues (Sync + Act) ----
            acat = const.tile([R2, DIN], F16)
            nc.scalar.dma_start(acat[:], A_cat)
            bcatt = const.tile([R2, DOUT], F16)
            nc.scalar.dma_start(bcatt[:], B_catT)
            bias_sb = const.tile([P, DOUT], F32)

            # x.T tiles: [128, k-tile, s-chunk 512] fp16, one DMA per s-chunk
            xts = [xt_pool.tile([P, KT, OTILE], F16, name=f"xt{sc}")
                   for sc in range(SC)]

            def issue_x_dma(sc, eng=None):
                # sc=0 rides the Activation DGE queue so its 1MB transfer
                # runs in parallel with wet0 on the Sync queue at startup
                (eng or nc.sync).dma_start(
                    xts[sc][:],
                    xT[:, ts(sc, OTILE)].rearrange("(kt p) s -> p kt s", p=P))

            def issue_wet_dma(ot, split=False):
                # DMA W.T chunk for o-tile `ot` into a fresh ring buffer.
                # split=True issues two half transfers so the k-tile 0..3
                # half lands earlier (startup critical path).
                w = wet_pool.tile([P, KT, OTILE], F16, tag="wet",
                                  name=f"wet{ot}")
                src = WT[:, ts(ot, OTILE)].rearrange("(kt p) o -> p kt o", p=P)
                if split:
                    nc.sync.dma_start(w[:, 0:KT // 2, :], src[:, 0:KT // 2, :])
                    nc.sync.dma_start(w[:, KT // 2:, :], src[:, KT // 2:, :])
                else:
                    nc.sync.dma_start(w[:], src)
                return w

            def merge_one(wet, ot, kt):
                # merge LoRA into one W.T chunk: wet[kt] += acat.T @ bcatt
                pl = pl_pool.tile([P, OTILE], F32, tag="pl")
                nc.tensor.matmul(pl[:], acat[:, ts(kt, P)],
                                 bcatt[:, ts(ot, OTILE)],
                                 start=True, stop=True)
                nc.vector.tensor_tensor(wet[:, kt, :], wet[:, kt, :], pl[:],
                                        mybir.AluOpType.add)

            wet_cur = issue_wet_dma(0, split=True)
            issue_x_dma(0, eng=nc.scalar)
            nc.sync.dma_start(bias_sb[:, ts(0, OTILE)], bias[:, ts(0, OTILE)])
            wet_next = issue_wet_dma(1)
            for sc in range(1, SC):
                issue_x_dma(sc)

            # ---- fused, software-pipelined merge + main loop over o-tiles.
            # ot=0, st=0 interleaves each LoRA matmul 1:1 with the group's
            # accumulating matmuls so the PE chews through the startup merge
            # chain (LoRA mm -> DVE add -> dependent mm) without idling.
            for ot in range(OT):
                if ot + 2 < OT:
                    wet_fut = issue_wet_dma(ot + 2)
                if ot + 1 < OT:
                    nc.sync.dma_start(bias_sb[:, ts(ot + 1, OTILE)],
                                      bias[:, ts(ot + 1, OTILE)])
                # main: out[s, o] = x @ wet + bias
                for st in range(ST):
                    if st % 2 == 1 and ot + 1 < OT:
                        merge_one(wet_next, ot + 1, (st - 1) // 2)
                    sc, sp = st // 4, st % 4
                    po = po_pool.tile([P, OTILE], F32, tag="po")
                    for kt in range(KT):
                        if ot == 0 and st == 0:
                            merge_one(wet_cur, 0, kt)
                        nc.tensor.matmul(po[:], xts[sc][:, kt, ts(sp, P)],
                                         wet_cur[:, kt, :],
                                         start=(kt == 0), stop=(kt == KT - 1))
                    osb = out_pool.tile([P, OTILE], F32)
                    nc.vector.tensor_tensor(osb[:], po[:],
                                            bias_sb[:, ts(ot, OTILE)],
                                            mybir.AluOpType.add)
                    nc.scalar.dma_start(out[ts(st, P), ts(ot, OTILE)], osb[:])
                if ot + 1 < OT:
                    wet_cur = wet_next
                if ot + 2 < OT:
                    wet_next = wet_fut

    nc.compile()
    return nc


_NC_CACHE = None


def _get_nc():
    global _NC_CACHE
    if _NC_CACHE is None:
        _NC_CACHE = build_nc()
    return _NC_CACHE


def make_in_maps(x, W, b, global_A, global_B, local_A, local_B):
    x16 = np.asarray(x, dtype=np.float32).astype(np.float16)
    xT = np.ascontiguousarray(x16.transpose(0, 2, 1))          # [B, DIN, S]
    WT = np.ascontiguousarray(
        np.asarray(W, dtype=np.float32).T).astype(np.float16)  # [DIN, DOUT]
    bias = np.ascontiguousarray(
        np.broadcast_to(np.asarray(b, dtype=np.float32), (P, DOUT)))
    A_cat = np.ascontiguousarray(
        SCALE * np.concatenate([np.asarray(global_A), np.asarray(local_A)],
                               axis=0)).astype(np.float16)
    B_catT = np.ascontiguousarray(
        np.concatenate([np.asarray(global_B).T, np.asarray(local_B).T],
                       axis=0)).astype(np.float16)
    return [
        {"xT": xT[i], "WT": WT, "bias": bias, "A_cat": A_cat,
         "B_catT": B_catT}
        for i in range(N_CORES)
    ]


def kernel(x, W, b, global_A, global_B, local_A, local_B):
    nc = _get_nc()
    in_maps = make_in_maps(x, W, b, global_A, global_B, local_A, local_B)
    res = run_bass_kernel_spmd(nc, in_maps, list(range(N_CORES))).results
    return np.stack([res[i]["out"] for i in range(N_CORES)], axis=0)


# revision 31
# speedup vs baseline: 1.0540x; 1.0493x over previous
"""LoRALinear kernel for Trainium2 (8 NeuronCores, SPMD data-parallel).

Computes out = x @ W.T + b + SCALE*((x@gA.T)@gB.T + (x@lA.T)@lB.T)
  x: [8, 2048, 1024] f32, W: [4096, 1024], b: [4096]
  gA/lA: [8, 1024], gB/lB: [4096, 8]  ->  out: [8, 2048, 4096] f32

Strategy: one batch of x per core. Host marshals pure layout/dtype only
(no module FLOPs): x -> x.T fp16 per core, W -> W.T fp16, b broadcast to
[128, 4096] f32, LoRA adapters stacked/pre-scaled as in the reference
low-rank-first formulation (A_cat = SCALE*[gA;lA] fp16, B_catT =
[gB.T;lB.T] fp16).

Device does all the math in ONE fused pipeline over o-tiles of 512:
  1. merge: wet[k, o] = W.T chunk + A_cat.T @ B_catT chunk (rank-16
     matmul into PSUM, DVE add evicts onto the DMA'd W.T chunk in a
     3-deep ring). The merge of o-tile ot+1 is sprinkled between the
     s-tile groups of main(ot) so neither the PE nor the Vector FIFO
     ever stalls at an o-tile boundary.
  2. main: for each of 16 s-tiles, 8 fp16 matmuls accumulate
     psum[s, o] over k; DVE adds bias f32 during eviction; out DMAs
     ride the Activation DGE queue so input prefetch (Sync queue)
     never queues behind stores.
A short burst of dependency-free warmup matmuls flips the PE HAM clock
gate to full rate while the first DMAs stream in. The PE stream is
~1088 back-to-back N=512 fp16 matmuls (~235us at 2.4GHz) with no phase
boundaries.

fp16 operand rounding gives ~3e-4 absmax relative error vs the f32
reference; accumulation stays f32 in PSUM.
"""
import numpy as np
from contextlib import ExitStack

import concourse.bass as bass
import concourse.tile as tile
from concourse import bacc, mybir
from concourse.bass import ts, ds
from concourse.bass_utils import run_bass_kernel_spmd

F32 = mybir.dt.float32
F16 = mybir.dt.float16

N_CORES = 8
B, S, DIN, DOUT, R = 8, 2048, 1024, 4096, 8
SCALE = 16.0 / 8
R2 = 2 * R

P = 128            # partition tile
OTILE = 512        # matmul moving free dim (one PSUM bank of f32)
KT = DIN // P      # 8 k-tiles
OT = DOUT // OTILE # 8 o-tiles
ST = S // P        # 16 s-tiles
SC = S // OTILE    # 4 s-chunks of 512 for x DMA granularity
WARMUP_MM = 20     # HAM warmup matmuls: keep the PE busy from the engine
                   # preamble (~7.5us) until the startup DMAs land (~14us)
                   # so the real stream starts and stays at 2.4GHz


def build_nc():
    nc = bacc.Bacc("TRN2", target_bir_lowering=False, debug=False,
                   num_devices=N_CORES)
    xT = nc.dram_tensor("xT", [DIN, S], F16, kind="ExternalInput").ap()
    WT = nc.dram_tensor("WT", [DIN, DOUT], F16, kind="ExternalInput").ap()
    bias = nc.dram_tensor("bias", [P, DOUT], F32, kind="ExternalInput").ap()
    A_cat = nc.dram_tensor("A_cat", [64, DIN], F16, kind="ExternalInput").ap()
    B_catT = nc.dram_tensor("B_catT", [64, DOUT], F16, kind="ExternalInput").ap()
    out = nc.dram_tensor("out", [S, DOUT], F32, kind="ExternalOutput").ap()

    with tile.TileContext(nc) as tc:
        with ExitStack() as ctx:
            const = ctx.enter_context(tc.tile_pool(name="const", bufs=1))
            xt_pool = ctx.enter_context(tc.tile_pool(name="xt", bufs=1))
            wet_pool = ctx.enter_context(tc.tile_pool(name="wet", bufs=3))
            out_pool = ctx.enter_context(tc.tile_pool(name="outp", bufs=4))
            pl_pool = ctx.enter_context(tc.tile_pool(name="pl", bufs=2, space="PSUM"))
            po_pool = ctx.enter_context(tc.tile_pool(name="po", bufs=6, space="PSUM"))

            # ---- HAM warmup: dependency-free matmuls run while the first
            # DMAs stream, so real matmuls start at 2.4GHz, not 1.2 ----
            junk = const.tile([P, OTILE], F16)
            nc.vector.memset(junk[:], 1.0)
            for i in range(WARMUP_MM):
                pw = po_pool.tile([P, OTILE], F32, tag="po")
                nc.tensor.matmul(pw[:], junk[:, :P], junk[:],
                                 start=True, stop=True)

            # ---- constants (input DMAs ride the Sync DGE queue in program
            # order; outputs use the Activation DGE queue so input prefetch
            # never queues behind result stores). The LoRA operands come
            # replicated at partition offsets 0 and 32 so pairs of rank-16
            # matmuls can run concurrently in separate 32-row PE strips. ----
            acat = const.tile([2 * 32, DIN], F16)
            nc.sync.dma_start(acat[:], A_cat)
            bcatt = const.tile([2 * 32, DOUT], F16)
            nc.sync.dma_start(bcatt[:], B_catT)
            bias_sb = const.tile([P, DOUT], F32)

            # x.T tiles: [128, k-tile, s-chunk 512] fp16, one DMA per s-chunk
            xts = [xt_pool.tile([P, KT, OTILE], F16, name=f"xt{sc}")
                   for sc in range(SC)]

            def issue_x_dma(sc, eng=None, split=False):
                # sc=0 rides the Activation DGE queue, k-tile 0 slice first,
                # so the first main matmuls unblock as early as possible
                eng = eng or nc.sync
                src = xT[:, ts(sc, OTILE)].rearrange("(kt p) s -> p kt s", p=P)
                if split:
                    eng.dma_start(xts[sc][:, 0, :], src[:, 0, :])
                    eng.dma_start(xts[sc][:, 1:, :], src[:, 1:, :])
                else:
                    eng.dma_start(xts[sc][:], src)

            def issue_wet_dma(ot, split=False):
                # DMA W.T chunk for o-tile `ot` into a fresh ring buffer.
                # split=True issues the k-tile 0 slice first (startup path).
                w = wet_pool.tile([P, KT, OTILE], F16, tag="wet",
                                  name=f"wet{ot}")
                src = WT[:, ts(ot, OTILE)].rearrange("(kt p) o -> p kt o", p=P)
                if split:
                    nc.sync.dma_start(w[:, 0, :], src[:, 0, :])
                    nc.sync.dma_start(w[:, 1:, :], src[:, 1:, :])
                else:
                    nc.sync.dma_start(w[:], src)
                return w

            def merge_pair(wet, ot, q):
                # merge LoRA into two W.T chunks: wet[kt] += acat.T @ bcatt
                # for kt = 2q, 2q+1, as two concurrently-executing rank-16
                # matmuls in PE row strips 0-31 / 32-63 (tile_position row
                # tiling), then two DVE adds.
                kts = (2 * q, 2 * q + 1)
                pls = [pl_pool.tile([P, OTILE], F32, tag="pl", name=f"pl{j}")
                       for j in range(2)]
                for j in range(2):
                    nc.tensor.matmul(pls[j][:],
                                     acat[ds(32 * j, R2), ts(kts[j], P)],
                                     bcatt[ds(32 * j, R2), ts(ot, OTILE)],
                                     start=True, stop=True,
                                     tile_position=(32 * j, 0))
                for j in range(2):
                    nc.vector.tensor_tensor(wet[:, kts[j], :],
                                            wet[:, kts[j], :], pls[j][:],
                                            mybir.AluOpType.add)

            wet_cur = issue_wet_dma(0)
            issue_x_dma(0)
            nc.sync.dma_start(bias_sb[:, ts(0, OTILE)], bias[:, ts(0, OTILE)])
            wet_next = issue_wet_dma(1)
            for sc in range(1, SC):
                issue_x_dma(sc)
            for q in range(KT // 2):
                merge_pair(wet_cur, 0, q)

            # ---- fused, software-pipelined merge + main loop over o-tiles.
            # The merge of o-tile ot+1 (8 LoRA matmuls + DVE adds) is
            # sprinkled between the s-tile groups of main(ot), so the PE
            # never bursts >pl-ring queued LoRA tiles and the DVE adds
            # interleave with already-drained bias evictions. ----
            for ot in range(OT):
                if ot + 2 < OT:
                    wet_fut = issue_wet_dma(ot + 2)
                if ot + 1 < OT:
                    nc.sync.dma_start(bias_sb[:, ts(ot + 1, OTILE)],
                                      bias[:, ts(ot + 1, OTILE)])
                # main: out[s, o] = x @ wet + bias
                for st in range(ST):
                    if st % 4 == 1 and ot + 1 < OT:
                        merge_pair(wet_next, ot + 1, (st - 1) // 4)
                    sc, sp = st // 4, st % 4
                    po = po_pool.tile([P, OTILE], F32, tag="po")
                    for kt in range(KT):
                        nc.tensor.matmul(po[:], xts[sc][:, kt, ts(sp, P)],
                                         wet_cur[:, kt, :],
                                         start=(kt == 0), stop=(kt == KT - 1))
                    osb = out_pool.tile([P, OTILE], F32, tag="osb")
                    nc.vector.tensor_tensor(osb[:], po[:],
                                            bias_sb[:, ts(ot, OTILE)],
                                            mybir.AluOpType.add)
                    nc.scalar.dma_start(out[ts(st, P), ts(ot, OTILE)], osb[:])
                if ot + 1 < OT:
                    wet_cur = wet_next
                if ot + 2 < OT:
                    wet_next = wet_fut

    nc.compile()
    return nc


_NC_CACHE = None


def _get_nc():
    global _NC_CACHE
    if _NC_CACHE is None:
        _NC_CACHE = build_nc()
    return _NC_CACHE


def make_in_maps(x, W, b, global_A, global_B, local_A, local_B):
    x16 = np.asarray(x, dtype=np.float32).astype(np.float16)
    xT = np.ascontiguousarray(x16.transpose(0, 2, 1))          # [B, DIN, S]
    WT = np.ascontiguousarray(
        np.asarray(W, dtype=np.float32).T).astype(np.float16)  # [DIN, DOUT]
    bias = np.ascontiguousarray(
        np.broadcast_to(np.asarray(b, dtype=np.float32), (P, DOUT)))
    a_cat = (SCALE * np.concatenate(
        [np.asarray(global_A), np.asarray(local_A)], axis=0)
    ).astype(np.float16)
    b_catT = np.concatenate(
        [np.asarray(global_B).T, np.asarray(local_B).T],
        axis=0).astype(np.float16)
    # replicate at partition offsets 0 and 32 for PE row-strip packing
    A_cat = np.zeros((64, DIN), dtype=np.float16)
    A_cat[0:R2] = a_cat
    A_cat[32:32 + R2] = a_cat
    B_catT = np.zeros((64, DOUT), dtype=np.float16)
    B_catT[0:R2] = b_catT
    B_catT[32:32 + R2] = b_catT
    return [
        {"xT": xT[i], "WT": WT, "bias": bias, "A_cat": A_cat,
         "B_catT": B_catT}
        for i in range(N_CORES)
    ]


def kernel(x, W, b, global_A, global_B, local_A, local_B):
    nc = _get_nc()
    in_maps = make_in_maps(x, W, b, global_A, global_B, local_A, local_B)
    res = run_bass_kernel_spmd(nc, in_maps, list(range(N_CORES))).results
    return np.stack([res[i]["out"] for i in range(N_CORES)], axis=0)


# revision 32
# speedup vs baseline: 1.0560x; 1.0019x over previous
"""LoRALinear kernel for Trainium2 (8 NeuronCores, SPMD data-parallel).

Computes out = x @ W.T + b + SCALE*((x@gA.T)@gB.T + (x@lA.T)@lB.T)
  x: [8, 2048, 1024] f32, W: [4096, 1024], b: [4096]
  gA/lA: [8, 1024], gB/lB: [4096, 8]  ->  out: [8, 2048, 4096] f32

Strategy: one batch of x per core. Host marshals pure layout/dtype only
(no module FLOPs): x -> x.T fp16 per core, W -> W.T fp16, b broadcast to
[128, 4096] f32, LoRA adapters stacked/pre-scaled as in the reference
low-rank-first formulation (A_cat = SCALE*[gA;lA] fp16, B_catT =
[gB.T;lB.T] fp16).

Device does all the math in ONE fused pipeline over o-tiles of 512:
  1. merge: wet[k, o] = W.T chunk + A_cat.T @ B_catT chunk (rank-16
     matmul into PSUM, DVE add evicts onto the DMA'd W.T chunk in a
     3-deep ring). The merge of o-tile ot+1 is sprinkled between the
     s-tile groups of main(ot) so neither the PE nor the Vector FIFO
     ever stalls at an o-tile boundary.
  2. main: for each of 16 s-tiles, 8 fp16 matmuls accumulate
     psum[s, o] over k; DVE adds bias f32 during eviction; out DMAs
     ride the Activation DGE queue so input prefetch (Sync queue)
     never queues behind stores.
A short burst of dependency-free warmup matmuls flips the PE HAM clock
gate to full rate while the first DMAs stream in. The PE stream is
~1088 back-to-back N=512 fp16 matmuls (~235us at 2.4GHz) with no phase
boundaries.

fp16 operand rounding gives ~3e-4 absmax relative error vs the f32
reference; accumulation stays f32 in PSUM.
"""
import numpy as np
from contextlib import ExitStack

import concourse.bass as bass
import concourse.tile as tile
from concourse import bacc, mybir
from concourse.bass import ts, ds
from concourse.bass_utils import run_bass_kernel_spmd

F32 = mybir.dt.float32
F16 = mybir.dt.float16

N_CORES = 8
B, S, DIN, DOUT, R = 8, 2048, 1024, 4096, 8
SCALE = 16.0 / 8
R2 = 2 * R

P = 128            # partition tile
OTILE = 512        # matmul moving free dim (one PSUM bank of f32)
KT = DIN // P      # 8 k-tiles
OT = DOUT // OTILE # 8 o-tiles
ST = S // P        # 16 s-tiles
SC = S // OTILE    # 4 s-chunks of 512 for x DMA granularity
WARMUP_MM = 28     # HAM warmup matmuls: keep the PE busy from the engine
                   # preamble (~7.5us) through the DVE-paced merge(0) chain
                   # (~15us) so the real stream starts and stays at 2.4GHz


def build_nc():
    nc = bacc.Bacc("TRN2", target_bir_lowering=False, debug=False,
                   num_devices=N_CORES)
    xT = nc.dram_tensor("xT", [DIN, S], F16, kind="ExternalInput").ap()
    WT = nc.dram_tensor("WT", [DIN, DOUT], F16, kind="ExternalInput").ap()
    bias = nc.dram_tensor("bias", [P, DOUT], F32, kind="ExternalInput").ap()
    A_cat = nc.dram_tensor("A_cat", [64, DIN], F16, kind="ExternalInput").ap()
    B_catT = nc.dram_tensor("B_catT", [64, DOUT], F16, kind="ExternalInput").ap()
    out = nc.dram_tensor("out", [S, DOUT], F32, kind="ExternalOutput").ap()

    with tile.TileContext(nc) as tc:
        with ExitStack() as ctx:
            const = ctx.enter_context(tc.tile_pool(name="const", bufs=1))
            xt_pool = ctx.enter_context(tc.tile_pool(name="xt", bufs=1))
            wet_pool = ctx.enter_context(tc.tile_pool(name="wet", bufs=3))
            out_pool = ctx.enter_context(tc.tile_pool(name="outp", bufs=4))
            pl_pool = ctx.enter_context(tc.tile_pool(name="pl", bufs=2, space="PSUM"))
            po_pool = ctx.enter_context(tc.tile_pool(name="po", bufs=6, space="PSUM"))

            # ---- HAM warmup: dependency-free matmuls run while the first
            # DMAs stream, so real matmuls start at 2.4GHz, not 1.2 ----
            junk = const.tile([P, OTILE], F16)
            nc.vector.memset(junk[:], 1.0)
            for i in range(WARMUP_MM):
                pw = po_pool.tile([P, OTILE], F32, tag="po")
                nc.tensor.matmul(pw[:], junk[:, :P], junk[:],
                                 start=True, stop=True)

            # ---- constants (input DMAs ride the Sync DGE queue in program
            # order; outputs use the Activation DGE queue so input prefetch
            # never queues behind result stores). The LoRA operands come
            # replicated at partition offsets 0 and 32 so pairs of rank-16
            # matmuls can run concurrently in separate 32-row PE strips. ----
            acat = const.tile([2 * 32, DIN], F16)
            nc.sync.dma_start(acat[:], A_cat)
            bcatt = const.tile([2 * 32, DOUT], F16)
            nc.sync.dma_start(bcatt[:], B_catT)
            bias_sb = const.tile([P, DOUT], F32)

            # x.T tiles: [128, k-tile, s-chunk 512] fp16, one DMA per s-chunk
            xts = [xt_pool.tile([P, KT, OTILE], F16, name=f"xt{sc}")
                   for sc in range(SC)]

            def issue_x_dma(sc, eng=None, split=False):
                # sc=0 rides the Activation DGE queue, k-tile 0 slice first,
                # so the first main matmuls unblock as early as possible
                eng = eng or nc.sync
                src = xT[:, ts(sc, OTILE)].rearrange("(kt p) s -> p kt s", p=P)
                if split:
                    eng.dma_start(xts[sc][:, 0, :], src[:, 0, :])
                    eng.dma_start(xts[sc][:, 1:, :], src[:, 1:, :])
                else:
                    eng.dma_start(xts[sc][:], src)

            def issue_wet_dma(ot, split=False):
                # DMA W.T chunk for o-tile `ot` into a fresh ring buffer.
                # split=True issues the k-tile 0 slice first (startup path).
                w = wet_pool.tile([P, KT, OTILE], F16, tag="wet",
                                  name=f"wet{ot}")
                src = WT[:, ts(ot, OTILE)].rearrange("(kt p) o -> p kt o", p=P)
                if split:
                    nc.sync.dma_start(w[:, 0, :], src[:, 0, :])
                    nc.sync.dma_start(w[:, 1:, :], src[:, 1:, :])
                else:
                    nc.sync.dma_start(w[:], src)
                return w

            def merge_pair(wet, ot, q):
                # merge LoRA into two W.T chunks: wet[kt] += acat.T @ bcatt
                # for kt = 2q, 2q+1, as two concurrently-executing rank-16
                # matmuls in PE row strips 0-31 / 32-63 (tile_position row
                # tiling), then two DVE adds.
                kts = (2 * q, 2 * q + 1)
                pls = [pl_pool.tile([P, OTILE], F32, tag="pl", name=f"pl{j}")
                       for j in range(2)]
                for j in range(2):
                    nc.tensor.matmul(pls[j][:],
                                     acat[ds(32 * j, R2), ts(kts[j], P)],
                                     bcatt[ds(32 * j, R2), ts(ot, OTILE)],
                                     start=True, stop=True,
                                     tile_position=(32 * j, 0))
                for j in range(2):
                    nc.vector.tensor_tensor(wet[:, kts[j], :],
                                            wet[:, kts[j], :], pls[j][:],
                                            mybir.AluOpType.add)

            wet_cur = issue_wet_dma(0)
            issue_x_dma(0)
            nc.sync.dma_start(bias_sb[:, ts(0, OTILE)], bias[:, ts(0, OTILE)])
            wet_next = issue_wet_dma(1)
            for sc in range(1, SC):
                issue_x_dma(sc)
            for q in range(KT // 2):
                merge_pair(wet_cur, 0, q)

            # ---- fused, software-pipelined merge + main loop over o-tiles.
            # The merge of o-tile ot+1 (8 LoRA matmuls + DVE adds) is
            # sprinkled between the s-tile groups of main(ot), so the PE
            # never bursts >pl-ring queued LoRA tiles and the DVE adds
            # interleave with already-drained bias evictions. ----
            for ot in range(OT):
                if ot + 2 < OT:
                    wet_fut = issue_wet_dma(ot + 2)
                if ot + 1 < OT:
                    nc.sync.dma_start(bias_sb[:, ts(ot + 1, OTILE)],
                                      bias[:, ts(ot + 1, OTILE)])
                # main: out[s, o] = x @ wet + bias
                for st in range(ST):
                    if st % 4 == 1 and ot + 1 < OT:
                        merge_pair(wet_next, ot + 1, (st - 1) // 4)
                    sc, sp = st // 4, st % 4
                    po = po_pool.tile([P, OTILE], F32, tag="po")
                    for kt in range(KT):
                        nc.tensor.matmul(po[:], xts[sc][:, kt, ts(sp, P)],
                                         wet_cur[:, kt, :],
                                         start=(kt == 0), stop=(kt == KT - 1))
                    osb = out_pool.tile([P, OTILE], F32, tag="osb")
                    nc.vector.tensor_tensor(osb[:], po[:],
                                            bias_sb[:, ts(ot, OTILE)],
                                            mybir.AluOpType.add)
                    nc.scalar.dma_start(out[ts(st, P), ts(ot, OTILE)], osb[:])
                if ot + 1 < OT:
                    wet_cur = wet_next
                if ot + 2 < OT:
                    wet_next = wet_fut

    nc.compile()
    return nc


_NC_CACHE = None


def _get_nc():
    global _NC_CACHE
    if _NC_CACHE is None:
        _NC_CACHE = build_nc()
    return _NC_CACHE


def make_in_maps(x, W, b, global_A, global_B, local_A, local_B):
    x16 = np.asarray(x, dtype=np.float32).astype(np.float16)
    xT = np.ascontiguousarray(x16.transpose(0, 2, 1))          # [B, DIN, S]
    WT = np.ascontiguousarray(
        np.asarray(W, dtype=np.float32).T).astype(np.float16)  # [DIN, DOUT]
    bias = np.ascontiguousarray(
        np.broadcast_to(np.asarray(b, dtype=np.float32), (P, DOUT)))
    a_cat = (SCALE * np.concatenate(
        [np.asarray(global_A), np.asarray(local_A)], axis=0)
    ).astype(np.float16)
    b_catT = np.concatenate(
        [np.asarray(global_B).T, np.asarray(local_B).T],
        axis=0).astype(np.float16)
    # replicate at partition offsets 0 and 32 for PE row-strip packing
    A_cat = np.zeros((64, DIN), dtype=np.float16)
    A_cat[0:R2] = a_cat
    A_cat[32:32 + R2] = a_cat
    B_catT = np.zeros((64, DOUT), dtype=np.float16)
    B_catT[0:R2] = b_catT
    B_catT[32:32 + R2] = b_catT
    return [
        {"xT": xT[i], "WT": WT, "bias": bias, "A_cat": A_cat,
         "B_catT": B_catT}
        for i in range(N_CORES)
    ]


def kernel(x, W, b, global_A, global_B, local_A, local_B):
    nc = _get_nc()
    in_maps = make_in_maps(x, W, b, global_A, global_B, local_A, local_B)
    res = run_bass_kernel_spmd(nc, in_maps, list(range(N_CORES))).results
    return np.stack([res[i]["out"] for i in range(N_CORES)], axis=0)
